# revision 1
# baseline (speedup 1.0000x reference)
import os
import numpy as np

# Model dims (hardcoded per spec: nn_BOPN_Model_45380624449999)
E = 256; H = 16; D = 16; FF = 512; L = 5; B = 4; N = 256; EPS = 1e-5

os.environ.setdefault("JAX_COMPILATION_CACHE_DIR", "/root/.cache/jax_bopn_cache")

LAST_HW_EXEC_NS = None


# ---------------- numpy fallback (always correct) ----------------

def _np_instance_norm(x, w, b):
    mu = x.mean(axis=0, keepdims=True)
    var = x.var(axis=0, keepdims=True)
    return (x - mu) / np.sqrt(var + EPS) * w + b


def _np_forward_one_batch(scaled, emb, P):
    inv_sqrt_d = np.float32(1.0 / np.sqrt(D))
    row, col = emb, emb
    scaledT = scaled.T.copy()
    for i in range(L):
        outs = []
        for j, (r, c, mix) in enumerate(((row, col, scaled),
                                         (col, row, scaledT))):
            q = (r @ P["Wq"][i, j]).reshape(N, H, D)
            k = (c @ P["Wk"][i, j]).reshape(N, H, D)
            v = (c @ P["Wv"][i, j]).reshape(N, H, D)
            score = np.einsum('nhd,mhd->hnm', q, k) * inv_sqrt_d
            score = score + mix[None, :, :] * P["alpha"][i, j][:, None, None] \
                + P["beta"][i, j][:, None, None]
            score -= score.max(axis=-1, keepdims=True)
            ex = np.exp(score)
            w = ex / ex.sum(axis=-1, keepdims=True)
            out = np.einsum('hnm,mhd->nhd', w, v).reshape(N, H * D)
            mh = out @ P["Wcomb"][i, j] + P["bcomb"][i, j]
            o1 = _np_instance_norm(r + mh, P["n1w"][i, j], P["n1b"][i, j])
            ff = np.maximum(o1 @ P["W1"][i, j] + P["b1"][i, j], 0.0) \
                @ P["W2"][i, j] + P["b2"][i, j]
            outs.append(_np_instance_norm(o1 + ff, P["n2w"][i, j],
                                          P["n2b"][i, j]))
        row, col = outs
    return row, col


def _np_kernel(scaled, emb, P):
    rows, cols = [], []
    for b in range(B):
        r, c = _np_forward_one_batch(scaled[b], emb[b], P)
        rows.append(r); cols.append(c)
    return np.stack(rows), np.stack(cols)


# ---------------- jax-on-NeuronCore path ----------------

_JAX_FN = None


def _get_jax_fn():
    global _JAX_FN
    if _JAX_FN is not None:
        return _JAX_FN
    import jax
    import jax.numpy as jnp

    def instance_norm(x, w, b):
        mu = jnp.mean(x, axis=0, keepdims=True)
        var = jnp.var(x, axis=0, keepdims=True)
        return (x - mu) * jax.lax.rsqrt(var + EPS) * w + b

    def forward(scaled, emb, P):
        # one batch element: scaled [N,N], emb [N,E]
        inv_sqrt_d = jnp.float32(1.0 / np.sqrt(D))
        scaledT = scaled.T
        row, col = emb, emb
        for i in range(L):
            outs = []
            for j, (r, c, mix) in enumerate(((row, col, scaled),
                                             (col, row, scaledT))):
                q = (r @ P["Wq"][i, j]).reshape(N, H, D)
                k = (c @ P["Wk"][i, j]).reshape(N, H, D)
                v = (c @ P["Wv"][i, j]).reshape(N, H, D)
                score = jnp.einsum('nhd,mhd->hnm', q, k,
                                   preferred_element_type=jnp.float32)
                score = score * inv_sqrt_d
                score = score + mix[None, :, :] \
                    * P["alpha"][i, j][:, None, None] \
                    + P["beta"][i, j][:, None, None]
                w = jax.nn.softmax(score, axis=-1)
                out = jnp.einsum('hnm,mhd->nhd', w, v,
                                 preferred_element_type=jnp.float32)
                out = out.reshape(N, H * D)
                mh = out @ P["Wcomb"][i, j] + P["bcomb"][i, j]
                o1 = instance_norm(r + mh, P["n1w"][i, j], P["n1b"][i, j])
                ff = jax.nn.relu(o1 @ P["W1"][i, j] + P["b1"][i, j]) \
                    @ P["W2"][i, j] + P["b2"][i, j]
                outs.append(instance_norm(o1 + ff, P["n2w"][i, j],
                                          P["n2b"][i, j]))
            row, col = outs
        return row, col

    _JAX_FN = (jax.jit(forward),
               jax.jit(jax.vmap(forward, in_axes=(0, 0, None))))
    return _JAX_FN


def kernel(data, node_rand, Wnode, bnode, Wedge, bedge,
           Wq, Wk, Wv, Wcomb, bcomb, n1w, n1b,
           W1, b1, W2, b2, n2w, n2b, Wmix):
    global LAST_HW_EXEC_NS
    f32 = np.float32
    data = np.asarray(data, f32)
    node_rand = np.asarray(node_rand, f32)

    # per-batch global min-max scaling of data
    flat = data.reshape(B, -1)
    mn = flat.min(axis=1).reshape(B, 1, 1)
    mx = flat.max(axis=1).reshape(B, 1, 1)
    rng = mx - mn
    rng = np.where(rng == 0, f32(1.0), rng).astype(f32)
    scaled = ((data - mn) / rng).astype(f32)        # [B,N,N]

    # edge tensor is rank-1 (edge[b,n,m,:] = scaled*Wedge + bedge), so the
    # mixed-score einsum collapses to scaled[b,n,m]*alpha[l,j,h] + beta[l,j,h]
    Wmix_ = np.asarray(Wmix, f32)
    alpha = np.einsum('e,ljeh->ljh', np.asarray(Wedge, f32)[0], Wmix_)
    beta = np.einsum('e,ljeh->ljh', np.asarray(bedge, f32), Wmix_)

    emb = (node_rand @ np.asarray(Wnode, f32)
           + np.asarray(bnode, f32)).astype(f32)    # [B,N,E]

    P = {
        "Wq": np.asarray(Wq, f32), "Wk": np.asarray(Wk, f32),
        "Wv": np.asarray(Wv, f32), "Wcomb": np.asarray(Wcomb, f32),
        "bcomb": np.asarray(bcomb, f32), "n1w": np.asarray(n1w, f32),
        "n1b": np.asarray(n1b, f32), "W1": np.asarray(W1, f32),
        "b1": np.asarray(b1, f32), "W2": np.asarray(W2, f32),
        "b2": np.asarray(b2, f32), "n2w": np.asarray(n2w, f32),
        "n2b": np.asarray(n2b, f32), "alpha": alpha, "beta": beta,
    }

    try:
        import time
        import jax
        devs = jax.devices()
        nd = min(B, len(devs))
        fn, fn_v = _get_jax_fn()
        try:
            # single-dispatch path: all B batches in one vmapped call on one
            # core — one host round-trip instead of four
            d0 = devs[0]
            Pd0 = {k: jax.device_put(v, d0) for k, v in P.items()}
            sd0 = jax.device_put(scaled, d0)
            ed0 = jax.device_put(emb, d0)
            r0, c0 = fn_v(sd0, ed0, Pd0)
            rows_v = np.asarray(r0)
            cols_v = np.asarray(c0)
            t0 = time.perf_counter()
            r1, c1 = fn_v(sd0, ed0, Pd0)
            r1.block_until_ready(); c1.block_until_ready()
            LAST_HW_EXEC_NS = (time.perf_counter() - t0) * 1e9
            return rows_v, cols_v
        except Exception:
            pass
        # shard batch across NeuronCores; params transferred once per device
        Pds = [{k: jax.device_put(v, devs[d]) for k, v in P.items()}
               for d in range(nd)]
        args = [(jax.device_put(scaled[b], devs[b % nd]),
                 jax.device_put(emb[b], devs[b % nd]),
                 Pds[b % nd]) for b in range(B)]
        futs = [fn(*a) for a in args]  # async dispatch, one per core
        rows = [np.asarray(r) for r, _ in futs]
        cols = [np.asarray(c) for _, c in futs]
        # warm timed pass (inputs already device-resident) for the HW number
        try:
            t0 = time.perf_counter()
            outs = [fn(*a) for a in args]
            for r, c in outs:
                r.block_until_ready(); c.block_until_ready()
            LAST_HW_EXEC_NS = (time.perf_counter() - t0) * 1e9
        except Exception:
            pass
        return np.stack(rows), np.stack(cols)
    except Exception:
        return _np_kernel(scaled, emb, P)


if __name__ == "__main__":
    # smoke test with random inputs
    rng_ = np.random.default_rng(0)
    out = kernel(
        data=rng_.normal(size=(B, N, N)).astype(np.float32),
        node_rand=rng_.random((B, N, 1), dtype=np.float32),
        Wnode=rng_.normal(size=(1, E)).astype(np.float32) * 0.05,
        bnode=np.zeros(E, np.float32),
        Wedge=rng_.normal(size=(1, E)).astype(np.float32) * 0.05,
        bedge=np.zeros(E, np.float32),
        Wq=rng_.normal(size=(L, 2, E, H * D)).astype(np.float32) * 0.05,
        Wk=rng_.normal(size=(L, 2, E, H * D)).astype(np.float32) * 0.05,
        Wv=rng_.normal(size=(L, 2, E, H * D)).astype(np.float32) * 0.05,
        Wcomb=rng_.normal(size=(L, 2, H * D, E)).astype(np.float32) * 0.05,
        bcomb=np.zeros((L, 2, E), np.float32),
        n1w=np.ones((L, 2, E), np.float32), n1b=np.zeros((L, 2, E), np.float32),
        W1=rng_.normal(size=(L, 2, E, FF)).astype(np.float32) * 0.05,
        b1=np.zeros((L, 2, FF), np.float32),
        W2=rng_.normal(size=(L, 2, FF, E)).astype(np.float32) * 0.05,
        b2=np.zeros((L, 2, E), np.float32),
        n2w=np.ones((L, 2, E), np.float32), n2b=np.zeros((L, 2, E), np.float32),
        Wmix=rng_.normal(size=(L, 2, E, H)).astype(np.float32) * 0.05,
    )
    print("shapes:", out[0].shape, out[1].shape,
          "HW ns:", LAST_HW_EXEC_NS)

